# revision 36
# baseline (speedup 1.0000x reference)
"""Trainium2 Bass kernel for nn_BAMM (pooled self-attention block + residual).

Reference computation (per batch sample, B=8 sharded 1/core over 8 cores):
  x  = avg_pool4(input)          [512, 32, 32] -> flat [512, 1024]
  y  = avg_pool4(c2)
  q  = Wq @ x + bq               [128, 1024]
  k  = Wk @ y + bk               [128, 1024]
  v  = Wv @ y + bv               [512, 1024]
  E  = (q^T k) / sqrt(128)       [1024, 1024]
  A  = softmax(E, axis=-1)
  o  = v @ A^T                   [512, 1024]
  out = upsample4(o) + c2        [512, 128, 128]

I/O strategy (HBM traffic is the roofline: 358 GB/s per core):
  - input is shipped as fp8 (e3m4) in natural [C, H*W] layout: 8 MB read.
    It only feeds q, whose influence on the output is tiny (attention
    deviations contribute ~2e-4 to an output of unit scale), so 4 mantissa
    bits are far more than enough.
  - c2 is shipped as bf16 in a "pool-plane hybrid" layout
    [C, nb(8), k(16), n128] where k = (i,j) indexes the 16 elements of each
    4x4 pooling window and n128 = (a, w') the 128 pooled positions of block
    nb. One read (16 MB); the full tensor stays resident in SBUF so the
    residual needs no second read.
  - out is written bf16 in the same hybrid layout (16 MB); the host
    un-permutes and upcasts. Total 40 MB/core vs 120 MB for the f32 version.

Compute mapping (v3: every PE matmul pays ~88 ns of serial LDWEIGHTS, so
minimize matmul count and keep pooling off the PE):
  - q: pooling fused into the PE: 16 offset-strided accumulating matmuls of
    512 columns per channel tile read the full-res fp8 input directly (wq
    has scale/16 folded in).
  - y-pool: contiguous bf16 tensor_tensor fold-adds on DVE (2x mode;
    k-planes are contiguous in the hybrid layout), at nb-pair granularity.
  - attention: eT[m,n] = k_blk^T q per m-block chases the c2 stream;
    softmax denominators via ones-matmul column sums; exp on ACT (energies
    are O(0.1), no max subtraction needed). bv is folded in after the
    normalize as a per-partition ACT bias (o += bv * colsum * recip = bv).
  - residual: out-plane[k] = c2-plane[k] + onrm, contiguous TT adds per
    (channel tile, half), in place in the SBUF cache, then 2 MB stores.
"""

import sys
import types

import numpy as np

import bass_rust

import concourse.bass as bass
import concourse.tile as tile
from concourse import mybir
from concourse.bass_utils import run_bass_kernel_spmd
from concourse.vector_clock import ScopedClock


class _TileContextPatched(tile.TileContext):
    """Work around a walrus sync-wait-count limit: the stock kernel-tail
    InstDrain carries every outstanding sem wait; this walrus build rejects
    more than one sync wait on a Drain. Spread the surplus across nofuse NOPs.
    """

    def _drain_and_barrier(self, tick_clock, wait_clock):
        nc = self.nc
        drain_inst = nc.sync.drain()
        wait_clock.add_sem_waits(
            drain_inst.ins, ScopedClock({None: tick_clock.global_clock})
        )
        si = drain_inst.ins.sync_info
        if si is not None and si.on_wait and len(si.on_wait) > 1:
            waits = list(si.on_wait)
            si.on_wait = waits[:1]
            for i in range(1, len(waits)):
                nop = nc.sync.nop(nofuse=True)
                nop.ins.sync_info = bass_rust.SyncInfo(
                    on_wait=waits[i:i + 1], on_update=[]
                )

        nc.all_engine_barrier()
        assert self.sems is not None
        popped = nc._tile_sem_poison_stack.pop()
        assert popped is self._sem_poison
        nc.clear_and_free_semaphores(list(self.sems.allocated().values()))
        nc.all_engine_barrier()

F32 = mybir.dt.float32
BF16 = mybir.dt.bfloat16
FP8 = mybir.dt.float8e4  # e4m3 (DoubleRow perf mode requires e4/e5)
QALPHA = float(2 ** 14)  # q rescale so fp8 wq lands in e4m3's normal range

_MW_COUNTER = [0]


def _split_multi_waits(nc, max_waits=1):
    """This walrus build encodes at most one sync wait per instruction.
    Hoist surplus waits onto same-engine NoOps inserted just before the
    over-subscribed instruction (engine programs execute in order, so the
    NoOps block the engine until every wait is satisfied)."""
    for f in nc.m.functions:
        for bb in f.blocks:
            new_list = []
            for ins in bb.instructions:
                si = ins.sync_info
                if si is not None and si.on_wait and len(si.on_wait) > max_waits:
                    waits = list(si.on_wait)
                    extras, keep = waits[:-max_waits], waits[-max_waits:]
                    for w in extras:
                        _MW_COUNTER[0] += 1
                        nop = bass_rust.InstNoOp(
                            name=f"I-mw{_MW_COUNTER[0]}", engine=ins.engine
                        )
                        nop.sync_info = bass_rust.SyncInfo(
                            on_wait=[w], on_update=[]
                        )
                        new_list.append(nop)
                    si.on_wait = keep
                new_list.append(ins)
            bb.instructions[:] = new_list

P = 128          # partitions
C = 512          # channels
CT = C // P      # 4 channel tiles
H = 128          # input spatial
DS = 4           # pool factor
NPIX = H * H     # 16384
N2 = 1024        # pooled positions
NB = 8           # position blocks of 128
MT = 8           # m-tiles (= NB)
CK = 128         # q/k channels


def _install_ntff_shim():
    """Register the axon NTFF profile hook if the image's antenv lacks it."""
    try:
        import antenv.axon_hooks  # noqa: F401
        return
    except ImportError:
        pass
    try:
        from trn_agent_boot.trn_boot import _ntff_profile_via_ctypes
        hook = _ntff_profile_via_ctypes("/opt/axon/libaxon_pjrt.so")
        m = types.ModuleType("antenv.axon_hooks")
        m.get_axon_ntff_profile_hook = lambda: hook
        sys.modules["antenv.axon_hooks"] = m
    except Exception:
        pass


def build_nc(split_waits=True):
    nc = bass.Bass()

    inp = nc.declare_dram_parameter("inp", [P, CT, NPIX], FP8, isOutput=False)
    c2h = nc.declare_dram_parameter("c2h", [C, NPIX], BF16, isOutput=False)
    # host-preprocessed weights (see prep_weights for the scale folding)
    wq = nc.declare_dram_parameter("wq", [P, CT, CK], FP8, isOutput=False)
    wk = nc.declare_dram_parameter("wk", [C, CK], BF16, isOutput=False)
    wv = nc.declare_dram_parameter("wv", [C, C], BF16, isOutput=False)
    bq = nc.declare_dram_parameter("bq", [CK, 1], F32, isOutput=False)  # * scale
    bk = nc.declare_dram_parameter("bk", [CK, 1], F32, isOutput=False)
    bv4 = nc.declare_dram_parameter("bv4", [P, CT], F32, isOutput=False)
    out = nc.declare_dram_parameter("outp", [C, NPIX], BF16, isOutput=True)

    with _TileContextPatched(nc) as tc:
        with nc.allow_low_precision(
            reason="bf16/fp8 attention path sits far inside the 2e-2 gate"
        ):
            _emit(nc, tc, inp, c2h, wq, wk, wv, bq, bk, bv4, out)
    if split_waits:
        _split_multi_waits(nc)
    return nc


def _emit(nc, tc, inp, c2h, wq, wk, wv, bq, bk, bv4, out):
    from contextlib import ExitStack

    Exp = mybir.ActivationFunctionType.Exp

    ctx = ExitStack()
    with ctx:
        const = ctx.enter_context(tc.tile_pool(name="const", bufs=1))
        feat = ctx.enter_context(tc.tile_pool(name="feat", bufs=1))
        stream = ctx.enter_context(tc.tile_pool(name="stream", bufs=3))
        psum = ctx.enter_context(tc.tile_pool(name="psum", bufs=1, space="PSUM"))

        # ---- constants: wq/bq must be dispatched ahead of everything (the
        # q matmuls start within a few us); the rest are not needed until
        # the attention phase.
        wq_sb = [const.tile([P, 2, CK], FP8, tag=f"wq{i}", name=f"wq{i}")
                 for i in range(2)]
        wk_sb = [const.tile([P, CK], BF16, tag=f"wk{i}", name=f"wk{i}") for i in range(CT)]
        wv_sb = [const.tile([P, C], BF16, tag=f"wv{i}", name=f"wv{i}") for i in range(CT)]
        bq_sb = const.tile([P, 1], F32, tag="bq")
        bk_sb = const.tile([P, 1], F32, tag="bk")
        bv_sb = const.tile([P, CT], F32, tag="bv")
        ones_sb = const.tile([P, P], BF16, tag="ones")
        for g in range(2):
            nc.scalar.dma_start(out=wq_sb[g][:], in_=wq[:, 2 * g:2 * g + 2, :])
        nc.scalar.dma_start(out=bq_sb[:], in_=bq[:])
        nc.vector.memset(ones_sb[:], 1.0)

        # ---- persistent tiles ----
        c2call = feat.tile([P, CT * NPIX], BF16, tag="c2c")
        c2c = [c2call[:, i * NPIX:(i + 1) * NPIX] for i in range(CT)]
        yf = [feat.tile([P, N2], BF16, tag=f"yf{i}", name=f"yf{i}") for i in range(CT)]
        q_sb = feat.tile([P, N2], BF16, tag="q")
        k_sb = feat.tile([P, N2], BF16, tag="k")
        vt_sb = [feat.tile([P, C], BF16, tag=f"vt{i}", name=f"vt{i}") for i in range(MT)]
        et_sb = [feat.tile([P, N2], BF16, tag=f"et{i}", name=f"et{i}") for i in range(MT)]
        recip = feat.tile([P, N2], BF16, tag="recip")

        for i in range(CT):
            nc.scalar.dma_start(out=wk_sb[i][:], in_=wk[i * P:(i + 1) * P, :])
            nc.scalar.dma_start(out=wv_sb[i][:], in_=wv[i * P:(i + 1) * P, :])
        nc.scalar.dma_start(out=bk_sb[:], in_=bk[:])
        nc.scalar.dma_start(out=bv_sb[:], in_=bv4[:, :])

        # ---- inp stream + fused pool-q matmuls (SP ring), with k/v blocks
        # interleaved between q-chunk groups so the post-q PE chain is just
        # energies + column sums.
        # s outer / ct inner so each 512-col psum accumulation group finishes
        # before the next group in the same bank starts (the start flag
        # clears has_written bits bank-wide).
        qp = psum.tile([P, N2], F32, tag="big", bufs=1)

        def emit_k(nb):
            blk = slice(nb * P, (nb + 1) * P)
            kp = psum.tile([P, 512], F32, tag="small", bufs=1, name="kp")
            for ct in range(CT):
                nc.tensor.matmul(kp[:, 0:128], wk_sb[ct][:], yf[ct][:, blk],
                                 start=(ct == 0), stop=(ct == CT - 1))
            nc.scalar.add(k_sb[:, blk], kp[:, 0:128], bk_sb[:])

        def emit_v(nb):
            blk = slice(nb * P, (nb + 1) * P)
            vp = psum.tile([P, 512], F32, tag="vt", bufs=1, name="vp")
            for ct in range(CT):
                nc.tensor.matmul(vp[:], yf[ct][:, blk], wv_sb[ct][:],
                                 start=(ct == 0), stop=(ct == CT - 1))
            nc.scalar.copy(out=vt_sb[nb][:], in_=vp[:])

        for s in range(4):
            for g in range(2):
                ich = stream.tile([P, 8192], FP8, tag="ich", bufs=2, name="ich")
                # inp rides the SP ring alone: its WAR-gated chunks must
                # not block the c2 slab queue (each ring executes FIFO).
                nc.sync.dma_start(
                    out=ich[:],
                    in_=inp[:, 2 * g:2 * g + 2, s * 4096:(s + 1) * 4096])
                v = ich[:].rearrange("p (t a i w j) -> p t i j a w",
                                     t=2, a=8, i=4, w=32, j=4)
                for ij in range(16):
                    i4, j4 = ij // 4, ij % 4
                    nc.tensor.matmul(
                        qp[:, s * 256:(s + 1) * 256], wq_sb[g][:],
                        v[:, :, i4, j4, :, :],
                        perf_mode=mybir.MatmulPerfMode.DoubleRow,
                        start=(g == 0 and ij == 0),
                        stop=(g == 1 and ij == 15),
                        skip_group_check=True)
            if s == 1:
                nc.scalar.add(q_sb[:, 0:512], qp[:, 0:512], bq_sb[:])

        # ---- c2 cache loads: same SP HWDGE ring as the inp chunks, queued
        # BEHIND them so the small inp stream drains at full HBM rate first
        # (q gates all attention work); the slab queue then streams at line
        # rate with no sequencer compute behind it.
        # first half of c2 streams on the (otherwise idle) SWDGE ring from
        # t=0, concurrent with the inp stream; second half rides the SP ring
        # behind the inp chunks. Each ring's FIFO throughput is ~250 GB/s,
        # so the split is what reaches the HBM rate.
        for nbp in range(4):
            cols = slice(nbp * 4096, (nbp + 1) * 4096)
            for ct in range(CT):
                eng = nc.gpsimd if nbp < 2 else nc.sync
                eng.dma_start(out=c2c[ct][:, cols],
                              in_=c2h[ct * P:(ct + 1) * P, cols])

        # ---- c2 pooling: DVE fold-adds at nb-pair granularity, emitted
        # first so the DVE chases the slab DMAs with no PE dependency.
        for nbp in range(4):
            for ct in range(CT):
                sl2 = c2c[ct].rearrange("p (b x) -> p b x", b=NB)
                s3 = sl2[:, 2 * nbp:2 * nbp + 2, :]
                ftmp = feat.tile([P, 2048], BF16, tag="ftmp", bufs=2,
                                 name="ftmp")
                f2 = ftmp[:].rearrange("p (b x) -> p b x", b=2)
                nc.vector.tensor_add(f2[:, :, :], s3[:, :, 0:1024],
                                     s3[:, :, 1024:2048])
                nc.vector.tensor_add(f2[:, :, 0:512], f2[:, :, 0:512],
                                     f2[:, :, 512:1024])
                nc.vector.tensor_add(f2[:, :, 0:256], f2[:, :, 0:256],
                                     f2[:, :, 256:512])
                yv = yf[ct][:, nbp * 256:(nbp + 1) * 256].rearrange(
                    "p (b x) -> p b x", b=2)
                nc.vector.tensor_add(yv, f2[:, :, 0:128], f2[:, :, 128:256])

        # ---- attention phase: per position-block k/v -> energy -> exp ->
        # colsum, all chasing the c2 stream (q is done early).
        sp = [psum.tile([P, 512], F32, tag=f"sp{nh}", name=f"sp{nh}", bufs=1)
              for nh in range(2)]

        def emit_colsums(nb):
            for nh in range(2):
                nc.tensor.matmul(
                    sp[nh][:], ones_sb[:],
                    et_sb[nb][:, nh * 512:(nh + 1) * 512],
                    start=(nb == 0), stop=(nb == NB - 1),
                    skip_group_check=True)

        # pass 1 (first q half): k/v blocks + energies chase the c2 stream;
        # pass 2 (second q half) runs as a fast burst once the stream ends.
        # The energy psum rotates between its two banks so PE matmul (nb+1)
        # overlaps ACT exp (nb).
        epb = psum.tile([P, N2], F32, tag="ep", bufs=1, name="epb")

        def emit_e(nb, nh):
            blk = slice(nb * P, (nb + 1) * P)
            reg = slice((nb % 2) * 512, (nb % 2) * 512 + 512)
            nc.tensor.matmul(epb[:, reg],
                             k_sb[:, blk], q_sb[:, nh * 512:(nh + 1) * 512],
                             start=True, stop=True)
            nc.scalar.activation(
                out=et_sb[nb][:, nh * 512:(nh + 1) * 512], in_=epb[:, reg],
                func=Exp)
            nc.tensor.matmul(
                sp[nh][:], ones_sb[:], et_sb[nb][:, nh * 512:(nh + 1) * 512],
                start=(nb == 0), stop=(nb == NB - 1), skip_group_check=True)

        for nb in range(NB):
            emit_k(nb)
            emit_e(nb, 0)
            emit_v(nb)
        nc.scalar.add(q_sb[:, 512:1024], qp[:, 512:1024], bq_sb[:])
        for nb in range(NB):
            emit_e(nb, 1)

        # 1/x as exp(-ln(x)) on the (idle) ACT engine: ~2 us total vs 6.6 us
        # for DVE's iterative-divide reciprocal, and off the DVE tail path.
        for nh in range(2):
            lnp = psum.tile([P, 512], F32, tag="small", bufs=1, name="lnp")
            nc.scalar.activation(out=lnp[:], in_=sp[nh][:],
                                 func=mybir.ActivationFunctionType.Ln)
            nc.scalar.activation(out=recip[:, nh * 512:(nh + 1) * 512],
                                 in_=lnp[:], func=Exp, scale=-1.0)

        # ---- tail: out bmm, normalize (+bv), residual + upsample, store ----
        for ct in range(CT):
            ops = psum.tile([P, N2], F32, tag="big", bufs=1, name="ops")
            for mt in range(MT):
                for nh in range(2):
                    nc.tensor.matmul(
                        ops[:, nh * 512:(nh + 1) * 512],
                        vt_sb[mt][:, ct * P:(ct + 1) * P],
                        et_sb[mt][:, nh * 512:(nh + 1) * 512],
                        start=(mt == 0), stop=(mt == MT - 1),
                        skip_group_check=True)
            onrm = feat.tile([P, N2], BF16, tag="onrm", bufs=2, name="onrm")
            for nh in range(2):
                half = slice(nh * 512, (nh + 1) * 512)
                nc.vector.tensor_mul(onrm[:, half], ops[:, half], recip[:, half])
                nc.scalar.add(onrm[:, half], onrm[:, half], bv_sb[:, ct:ct + 1])
                ov = (onrm[:, half].rearrange("p (b n) -> p b n", b=4)
                      .unsqueeze(2).broadcast_to([P, 4, 16, P]))
                cols = slice(nh * 8192, (nh + 1) * 8192)
                cv = c2c[ct][:, cols].rearrange("p (b k n) -> p b k n",
                                                b=4, k=16)
                nc.vector.tensor_add(cv, cv, ov)
                # stripe the out stream across the ACT and SP HWDGE rings
                # (both are idle by the tail) to beat the per-ring ceiling
                eng = nc.scalar if nh == 0 else nc.sync
                eng.dma_start(out=out[ct * P:(ct + 1) * P, cols],
                              in_=c2c[ct][:, cols])


_NC_CACHE = None


def _get_nc():
    global _NC_CACHE
    if _NC_CACHE is None:
        _install_ntff_shim()
        _NC_CACHE = build_nc()
    return _NC_CACHE


def prep_weights(Wq, bq, Wk, bk, Wv, bv):
    scale = np.float32(1.0 / np.sqrt(np.float32(CK)))
    sixteenth = np.float32(1.0 / 16.0)
    alpha = np.float32(QALPHA)
    import ml_dtypes
    bf16 = ml_dtypes.bfloat16
    fp8 = ml_dtypes.float8_e4m3
    # q is computed at alpha x scale so the fp8 wq values sit in e4m3's
    # normal range; k (and bk) carry the compensating 1/alpha.
    wq8 = (Wq.T * (scale * sixteenth * alpha)).astype(fp8)      # [C, CK]
    wq8 = wq8.reshape(CT, P, CK).transpose(1, 0, 2)             # [P, CT, CK]
    return {
        "wq": np.ascontiguousarray(wq8),
        "wk": np.ascontiguousarray((Wk.T * (sixteenth / alpha)).astype(bf16)),
        "wv": np.ascontiguousarray((Wv.T * sixteenth).astype(bf16)),
        "bq": np.ascontiguousarray((bq * (scale * alpha)).reshape(CK, 1),
                                   dtype=np.float32),
        "bk": np.ascontiguousarray((bk / alpha).reshape(CK, 1),
                                   dtype=np.float32),
        "bv4": np.ascontiguousarray(bv.reshape(CT, P).T, dtype=np.float32),
    }


def _pack_hybrid(x):
    """[B, C, 128, 128] f32 -> [B, C, 16384] bf16 in [nb, (i,j), (a,w')] order."""
    import ml_dtypes
    B = x.shape[0]
    xb = x.reshape(B, C, NB, 4, 4, 32, 4)          # [b, c, nb, a, i, w', j]
    xb = xb.transpose(0, 1, 2, 4, 6, 3, 5)         # [b, c, nb, i, j, a, w']
    return np.ascontiguousarray(xb.reshape(B, C, NPIX).astype(ml_dtypes.bfloat16))


def _unpack_hybrid(y):
    """[C, 16384] bf16 hybrid -> [C, 128, 128] f32 natural."""
    yb = np.asarray(y).astype(np.float32).reshape(C, NB, 4, 4, 4, 32)
    # dims: [c, nb, i, j, a, w'] -> [c, nb, a, i, w', j]
    yb = yb.transpose(0, 1, 4, 2, 5, 3)
    return yb.reshape(C, H, H)


def kernel(input, c2, Wq, bq, Wk, bk, Wv, bv, _trace=False):
    import ml_dtypes
    input = np.asarray(input, dtype=np.float32)
    c2 = np.asarray(c2, dtype=np.float32)
    w = prep_weights(
        np.asarray(Wq, np.float32), np.asarray(bq, np.float32),
        np.asarray(Wk, np.float32), np.asarray(bk, np.float32),
        np.asarray(Wv, np.float32), np.asarray(bv, np.float32),
    )
    B = input.shape[0]
    # [B, C, npix] -> [B, P, CT, npix]: partition p holds channels
    # {p, 128+p, 256+p, 384+p} so DoubleRow matmuls contract channel-tile
    # pairs out of one partition's free dim.
    inp8 = np.ascontiguousarray(
        input.reshape(B, CT, P, NPIX).transpose(0, 2, 1, 3)
        .astype(ml_dtypes.float8_e4m3))
    c2hyb = _pack_hybrid(c2)
    nc = _get_nc()
    in_maps = [
        {"inp": inp8[i], "c2h": c2hyb[i], **w}
        for i in range(B)
    ]
    res = run_bass_kernel_spmd(nc, in_maps, list(range(B)), trace=_trace)
    outp = np.stack([_unpack_hybrid(res.results[i]["outp"]) for i in range(B)])
    if _trace:
        kernel._last_result = res
    return outp


# revision 37
# speedup vs baseline: 1.1095x; 1.1095x over previous
"""Trainium2 Bass kernel for nn_BAMM (pooled self-attention block + residual).

Reference computation (per batch sample, B=8 sharded 1/core over 8 cores):
  x  = avg_pool4(input)          [512, 32, 32] -> flat [512, 1024]
  y  = avg_pool4(c2)
  q  = Wq @ x + bq               [128, 1024]
  k  = Wk @ y + bk               [128, 1024]
  v  = Wv @ y + bv               [512, 1024]
  E  = (q^T k) / sqrt(128)       [1024, 1024]
  A  = softmax(E, axis=-1)
  o  = v @ A^T                   [512, 1024]
  out = upsample4(o) + c2        [512, 128, 128]

I/O strategy (HBM traffic is the roofline: 358 GB/s per core):
  - input is shipped as fp8 (e3m4) in natural [C, H*W] layout: 8 MB read.
    It only feeds q, whose influence on the output is tiny (attention
    deviations contribute ~2e-4 to an output of unit scale), so 4 mantissa
    bits are far more than enough.
  - c2 is shipped as bf16 in a "pool-plane hybrid" layout
    [C, nb(8), k(16), n128] where k = (i,j) indexes the 16 elements of each
    4x4 pooling window and n128 = (a, w') the 128 pooled positions of block
    nb. One read (16 MB); the full tensor stays resident in SBUF so the
    residual needs no second read.
  - out is written bf16 in the same hybrid layout (16 MB); the host
    un-permutes and upcasts. Total 40 MB/core vs 120 MB for the f32 version.

Compute mapping (v3: every PE matmul pays ~88 ns of serial LDWEIGHTS, so
minimize matmul count and keep pooling off the PE):
  - q: pooling fused into the PE: 16 offset-strided accumulating matmuls of
    512 columns per channel tile read the full-res fp8 input directly (wq
    has scale/16 folded in).
  - y-pool: contiguous bf16 tensor_tensor fold-adds on DVE (2x mode;
    k-planes are contiguous in the hybrid layout), at nb-pair granularity.
  - attention: eT[m,n] = k_blk^T q per m-block chases the c2 stream;
    softmax denominators via ones-matmul column sums; exp on ACT (energies
    are O(0.1), no max subtraction needed). bv is folded in after the
    normalize as a per-partition ACT bias (o += bv * colsum * recip = bv).
  - residual: out-plane[k] = c2-plane[k] + onrm, contiguous TT adds per
    (channel tile, half), in place in the SBUF cache, then 2 MB stores.
"""

import sys
import types

import numpy as np

import bass_rust

import concourse.bass as bass
import concourse.tile as tile
from concourse import mybir
from concourse.bass_utils import run_bass_kernel_spmd
from concourse.vector_clock import ScopedClock


class _TileContextPatched(tile.TileContext):
    """Work around a walrus sync-wait-count limit: the stock kernel-tail
    InstDrain carries every outstanding sem wait; this walrus build rejects
    more than one sync wait on a Drain. Spread the surplus across nofuse NOPs.
    """

    def _drain_and_barrier(self, tick_clock, wait_clock):
        nc = self.nc
        drain_inst = nc.sync.drain()
        wait_clock.add_sem_waits(
            drain_inst.ins, ScopedClock({None: tick_clock.global_clock})
        )
        si = drain_inst.ins.sync_info
        if si is not None and si.on_wait and len(si.on_wait) > 1:
            waits = list(si.on_wait)
            si.on_wait = waits[:1]
            for i in range(1, len(waits)):
                nop = nc.sync.nop(nofuse=True)
                nop.ins.sync_info = bass_rust.SyncInfo(
                    on_wait=waits[i:i + 1], on_update=[]
                )

        nc.all_engine_barrier()
        assert self.sems is not None
        popped = nc._tile_sem_poison_stack.pop()
        assert popped is self._sem_poison
        nc.clear_and_free_semaphores(list(self.sems.allocated().values()))
        nc.all_engine_barrier()

F32 = mybir.dt.float32
BF16 = mybir.dt.bfloat16
FP8 = mybir.dt.float8e4  # e4m3 (DoubleRow perf mode requires e4/e5)
QALPHA = float(2 ** 14)  # q rescale so fp8 wq lands in e4m3's normal range

_MW_COUNTER = [0]


def _split_multi_waits(nc, max_waits=1):
    """This walrus build encodes at most one sync wait per instruction.
    Hoist surplus waits onto same-engine NoOps inserted just before the
    over-subscribed instruction (engine programs execute in order, so the
    NoOps block the engine until every wait is satisfied)."""
    for f in nc.m.functions:
        for bb in f.blocks:
            new_list = []
            for ins in bb.instructions:
                si = ins.sync_info
                if si is not None and si.on_wait and len(si.on_wait) > max_waits:
                    waits = list(si.on_wait)
                    extras, keep = waits[:-max_waits], waits[-max_waits:]
                    for w in extras:
                        _MW_COUNTER[0] += 1
                        nop = bass_rust.InstNoOp(
                            name=f"I-mw{_MW_COUNTER[0]}", engine=ins.engine
                        )
                        nop.sync_info = bass_rust.SyncInfo(
                            on_wait=[w], on_update=[]
                        )
                        new_list.append(nop)
                    si.on_wait = keep
                new_list.append(ins)
            bb.instructions[:] = new_list

P = 128          # partitions
C = 512          # channels
CT = C // P      # 4 channel tiles
H = 128          # input spatial
DS = 4           # pool factor
NPIX = H * H     # 16384
N2 = 1024        # pooled positions
NB = 8           # position blocks of 128
MT = 8           # m-tiles (= NB)
CK = 128         # q/k channels


def _install_ntff_shim():
    """Register the axon NTFF profile hook if the image's antenv lacks it."""
    try:
        import antenv.axon_hooks  # noqa: F401
        return
    except ImportError:
        pass
    try:
        from trn_agent_boot.trn_boot import _ntff_profile_via_ctypes
        hook = _ntff_profile_via_ctypes("/opt/axon/libaxon_pjrt.so")
        m = types.ModuleType("antenv.axon_hooks")
        m.get_axon_ntff_profile_hook = lambda: hook
        sys.modules["antenv.axon_hooks"] = m
    except Exception:
        pass


def build_nc(split_waits=True):
    nc = bass.Bass()

    inp = nc.declare_dram_parameter("inp", [P, CT, NPIX], FP8, isOutput=False)
    c2h = nc.declare_dram_parameter("c2h", [C, NPIX], BF16, isOutput=False)
    # host-preprocessed weights (see prep_weights for the scale folding)
    wq = nc.declare_dram_parameter("wq", [P, CT, CK], FP8, isOutput=False)
    wk = nc.declare_dram_parameter("wk", [C, CK], BF16, isOutput=False)
    wv = nc.declare_dram_parameter("wv", [C, C], BF16, isOutput=False)
    bq = nc.declare_dram_parameter("bq", [CK, 1], F32, isOutput=False)  # * scale
    bk = nc.declare_dram_parameter("bk", [CK, 1], F32, isOutput=False)
    bv4 = nc.declare_dram_parameter("bv4", [P, CT], F32, isOutput=False)
    out = nc.declare_dram_parameter("outp", [C, NPIX], BF16, isOutput=True)

    with _TileContextPatched(nc) as tc:
        with nc.allow_low_precision(
            reason="bf16/fp8 attention path sits far inside the 2e-2 gate"
        ):
            _emit(nc, tc, inp, c2h, wq, wk, wv, bq, bk, bv4, out)
    if split_waits:
        _split_multi_waits(nc)
    return nc


def _emit(nc, tc, inp, c2h, wq, wk, wv, bq, bk, bv4, out):
    from contextlib import ExitStack

    Exp = mybir.ActivationFunctionType.Exp

    ctx = ExitStack()
    with ctx:
        const = ctx.enter_context(tc.tile_pool(name="const", bufs=1))
        feat = ctx.enter_context(tc.tile_pool(name="feat", bufs=1))
        stream = ctx.enter_context(tc.tile_pool(name="stream", bufs=3))
        psum = ctx.enter_context(tc.tile_pool(name="psum", bufs=1, space="PSUM"))

        # ---- constants: wq/bq must be dispatched ahead of everything (the
        # q matmuls start within a few us); the rest are not needed until
        # the attention phase.
        wq_sb = [const.tile([P, 2, CK], FP8, tag=f"wq{i}", name=f"wq{i}")
                 for i in range(2)]
        wk_sb = [const.tile([P, CK], BF16, tag=f"wk{i}", name=f"wk{i}") for i in range(CT)]
        wv_sb = [const.tile([P, C], BF16, tag=f"wv{i}", name=f"wv{i}") for i in range(CT)]
        bq_sb = const.tile([P, 1], F32, tag="bq")
        bk_sb = const.tile([P, 1], F32, tag="bk")
        bv_sb = const.tile([P, CT], F32, tag="bv")
        ones_sb = const.tile([P, P], BF16, tag="ones")
        for g in range(2):
            nc.scalar.dma_start(out=wq_sb[g][:], in_=wq[:, 2 * g:2 * g + 2, :])
        nc.scalar.dma_start(out=bq_sb[:], in_=bq[:])
        nc.vector.memset(ones_sb[:], 1.0)

        # ---- persistent tiles ----
        c2call = feat.tile([P, CT * NPIX], BF16, tag="c2c")
        c2c = [c2call[:, i * NPIX:(i + 1) * NPIX] for i in range(CT)]
        yf = [feat.tile([P, N2], BF16, tag=f"yf{i}", name=f"yf{i}") for i in range(CT)]
        q_sb = feat.tile([P, N2], BF16, tag="q")
        k_sb = feat.tile([P, N2], BF16, tag="k")
        vt_sb = [feat.tile([P, C], BF16, tag=f"vt{i}", name=f"vt{i}") for i in range(MT)]
        et_sb = [feat.tile([P, N2], BF16, tag=f"et{i}", name=f"et{i}") for i in range(MT)]
        recip = feat.tile([P, N2], F32, tag="recip")

        for i in range(CT):
            nc.scalar.dma_start(out=wk_sb[i][:], in_=wk[i * P:(i + 1) * P, :])
            nc.scalar.dma_start(out=wv_sb[i][:], in_=wv[i * P:(i + 1) * P, :])
        nc.scalar.dma_start(out=bk_sb[:], in_=bk[:])
        nc.scalar.dma_start(out=bv_sb[:], in_=bv4[:, :])

        # ---- inp stream + fused pool-q matmuls (SP ring), with k/v blocks
        # interleaved between q-chunk groups so the post-q PE chain is just
        # energies + column sums.
        # s outer / ct inner so each 512-col psum accumulation group finishes
        # before the next group in the same bank starts (the start flag
        # clears has_written bits bank-wide).
        qp = psum.tile([P, N2], F32, tag="big", bufs=1)

        def emit_k(nb):
            blk = slice(nb * P, (nb + 1) * P)
            kp = psum.tile([P, 512], F32, tag="small", bufs=1, name="kp")
            for ct in range(CT):
                nc.tensor.matmul(kp[:, 0:128], wk_sb[ct][:], yf[ct][:, blk],
                                 start=(ct == 0), stop=(ct == CT - 1))
            nc.scalar.add(k_sb[:, blk], kp[:, 0:128], bk_sb[:])

        def emit_v(nb):
            blk = slice(nb * P, (nb + 1) * P)
            vp = psum.tile([P, 512], F32, tag="vt", bufs=1, name="vp")
            for ct in range(CT):
                nc.tensor.matmul(vp[:], yf[ct][:, blk], wv_sb[ct][:],
                                 start=(ct == 0), stop=(ct == CT - 1))
            nc.scalar.copy(out=vt_sb[nb][:], in_=vp[:])

        for s in range(4):
            for g in range(2):
                ich = stream.tile([P, 8192], FP8, tag="ich", bufs=2, name="ich")
                # stripe across the SP HWDGE and GPSIMD SWDGE rings: each
                # ring executes its DMAs FIFO (~250 GB/s ceiling), so one
                # ring alone cannot reach the 358 GB/s HBM rate.
                eng = nc.sync if (s * 2 + g) % 2 == 0 else nc.gpsimd
                eng.dma_start(
                    out=ich[:],
                    in_=inp[:, 2 * g:2 * g + 2, s * 4096:(s + 1) * 4096])
                v = ich[:].rearrange("p (t a i w j) -> p t i j a w",
                                     t=2, a=8, i=4, w=32, j=4)
                for ij in range(16):
                    i4, j4 = ij // 4, ij % 4
                    nc.tensor.matmul(
                        qp[:, s * 256:(s + 1) * 256], wq_sb[g][:],
                        v[:, :, i4, j4, :, :],
                        perf_mode=mybir.MatmulPerfMode.DoubleRow,
                        start=(g == 0 and ij == 0),
                        stop=(g == 1 and ij == 15),
                        skip_group_check=True)
        nc.scalar.add(q_sb[:], qp[:], bq_sb[:])

        # ---- c2 cache loads: same SP HWDGE ring as the inp chunks, queued
        # BEHIND them so the small inp stream drains at full HBM rate first
        # (q gates all attention work); the slab queue then streams at line
        # rate with no sequencer compute behind it.
        for nbp in range(4):
            cols = slice(nbp * 4096, (nbp + 1) * 4096)
            for ct in range(CT):
                eng = nc.sync if ct % 2 == 0 else nc.gpsimd
                eng.dma_start(out=c2c[ct][:, cols],
                              in_=c2h[ct * P:(ct + 1) * P, cols])

        # ---- c2 pooling: DVE fold-adds at nb-pair granularity, emitted
        # first so the DVE chases the slab DMAs with no PE dependency.
        for nbp in range(4):
            for ct in range(CT):
                sl2 = c2c[ct].rearrange("p (b x) -> p b x", b=NB)
                s3 = sl2[:, 2 * nbp:2 * nbp + 2, :]
                ftmp = feat.tile([P, 2048], BF16, tag="ftmp", bufs=2,
                                 name="ftmp")
                f2 = ftmp[:].rearrange("p (b x) -> p b x", b=2)
                nc.vector.tensor_add(f2[:, :, :], s3[:, :, 0:1024],
                                     s3[:, :, 1024:2048])
                nc.vector.tensor_add(f2[:, :, 0:512], f2[:, :, 0:512],
                                     f2[:, :, 512:1024])
                nc.vector.tensor_add(f2[:, :, 0:256], f2[:, :, 0:256],
                                     f2[:, :, 256:512])
                yv = yf[ct][:, nbp * 256:(nbp + 1) * 256].rearrange(
                    "p (b x) -> p b x", b=2)
                nc.vector.tensor_add(yv, f2[:, :, 0:128], f2[:, :, 128:256])

        # ---- attention phase: per position-block k/v -> energy -> exp ->
        # colsum, all chasing the c2 stream (q is done early).
        sp = [psum.tile([P, 512], F32, tag=f"sp{nh}", name=f"sp{nh}", bufs=1)
              for nh in range(2)]

        def emit_colsums(nb):
            for nh in range(2):
                nc.tensor.matmul(
                    sp[nh][:], ones_sb[:],
                    et_sb[nb][:, nh * 512:(nh + 1) * 512],
                    start=(nb == 0), stop=(nb == NB - 1),
                    skip_group_check=True)

        pend = None
        for nb in range(NB):
            blk = slice(nb * P, (nb + 1) * P)
            emit_k(nb)
            if pend is not None:
                emit_colsums(pend)
                pend = None
            epb = psum.tile([P, N2], F32, tag="ep", bufs=1, name="epb")
            nc.tensor.matmul(epb[:, 0:512], k_sb[:, blk], q_sb[:, 0:512],
                             start=True, stop=True)
            nc.tensor.matmul(epb[:, 512:1024], k_sb[:, blk], q_sb[:, 512:1024],
                             start=True, stop=True)
            nc.scalar.activation(out=et_sb[nb][:], in_=epb[:], func=Exp)
            emit_v(nb)
            pend = nb
        emit_colsums(pend)

        nc.vector.reciprocal(recip[:, 0:512], sp[0][:])
        nc.vector.reciprocal(recip[:, 512:1024], sp[1][:])

        # ---- tail: out bmm, normalize (+bv), residual + upsample, store ----
        for ct in range(CT):
            ops = psum.tile([P, N2], F32, tag="big", bufs=1, name="ops")
            for mt in range(MT):
                for nh in range(2):
                    nc.tensor.matmul(
                        ops[:, nh * 512:(nh + 1) * 512],
                        vt_sb[mt][:, ct * P:(ct + 1) * P],
                        et_sb[mt][:, nh * 512:(nh + 1) * 512],
                        start=(mt == 0), stop=(mt == MT - 1),
                        skip_group_check=True)
            onrm = feat.tile([P, N2], BF16, tag="onrm", bufs=2, name="onrm")
            for nh in range(2):
                half = slice(nh * 512, (nh + 1) * 512)
                nc.vector.tensor_mul(onrm[:, half], ops[:, half], recip[:, half])
                nc.scalar.add(onrm[:, half], onrm[:, half], bv_sb[:, ct:ct + 1])
                ov = (onrm[:, half].rearrange("p (b n) -> p b n", b=4)
                      .unsqueeze(2).broadcast_to([P, 4, 16, P]))
                cols = slice(nh * 8192, (nh + 1) * 8192)
                cv = c2c[ct][:, cols].rearrange("p (b k n) -> p b k n",
                                                b=4, k=16)
                nc.vector.tensor_add(cv, cv, ov)
                # stripe the out stream across the ACT and SP HWDGE rings
                # (both are idle by the tail) to beat the per-ring ceiling
                eng = nc.scalar if nh == 0 else nc.sync
                eng.dma_start(out=out[ct * P:(ct + 1) * P, cols],
                              in_=c2c[ct][:, cols])


_NC_CACHE = None


def _get_nc():
    global _NC_CACHE
    if _NC_CACHE is None:
        _install_ntff_shim()
        _NC_CACHE = build_nc()
    return _NC_CACHE


def prep_weights(Wq, bq, Wk, bk, Wv, bv):
    scale = np.float32(1.0 / np.sqrt(np.float32(CK)))
    sixteenth = np.float32(1.0 / 16.0)
    alpha = np.float32(QALPHA)
    import ml_dtypes
    bf16 = ml_dtypes.bfloat16
    fp8 = ml_dtypes.float8_e4m3
    # q is computed at alpha x scale so the fp8 wq values sit in e4m3's
    # normal range; k (and bk) carry the compensating 1/alpha.
    wq8 = (Wq.T * (scale * sixteenth * alpha)).astype(fp8)      # [C, CK]
    wq8 = wq8.reshape(CT, P, CK).transpose(1, 0, 2)             # [P, CT, CK]
    return {
        "wq": np.ascontiguousarray(wq8),
        "wk": np.ascontiguousarray((Wk.T * (sixteenth / alpha)).astype(bf16)),
        "wv": np.ascontiguousarray((Wv.T * sixteenth).astype(bf16)),
        "bq": np.ascontiguousarray((bq * (scale * alpha)).reshape(CK, 1),
                                   dtype=np.float32),
        "bk": np.ascontiguousarray((bk / alpha).reshape(CK, 1),
                                   dtype=np.float32),
        "bv4": np.ascontiguousarray(bv.reshape(CT, P).T, dtype=np.float32),
    }


def _pack_hybrid(x):
    """[B, C, 128, 128] f32 -> [B, C, 16384] bf16 in [nb, (i,j), (a,w')] order."""
    import ml_dtypes
    B = x.shape[0]
    xb = x.reshape(B, C, NB, 4, 4, 32, 4)          # [b, c, nb, a, i, w', j]
    xb = xb.transpose(0, 1, 2, 4, 6, 3, 5)         # [b, c, nb, i, j, a, w']
    return np.ascontiguousarray(xb.reshape(B, C, NPIX).astype(ml_dtypes.bfloat16))


def _unpack_hybrid(y):
    """[C, 16384] bf16 hybrid -> [C, 128, 128] f32 natural."""
    yb = np.asarray(y).astype(np.float32).reshape(C, NB, 4, 4, 4, 32)
    # dims: [c, nb, i, j, a, w'] -> [c, nb, a, i, w', j]
    yb = yb.transpose(0, 1, 4, 2, 5, 3)
    return yb.reshape(C, H, H)


def kernel(input, c2, Wq, bq, Wk, bk, Wv, bv, _trace=False):
    import ml_dtypes
    input = np.asarray(input, dtype=np.float32)
    c2 = np.asarray(c2, dtype=np.float32)
    w = prep_weights(
        np.asarray(Wq, np.float32), np.asarray(bq, np.float32),
        np.asarray(Wk, np.float32), np.asarray(bk, np.float32),
        np.asarray(Wv, np.float32), np.asarray(bv, np.float32),
    )
    B = input.shape[0]
    # [B, C, npix] -> [B, P, CT, npix]: partition p holds channels
    # {p, 128+p, 256+p, 384+p} so DoubleRow matmuls contract channel-tile
    # pairs out of one partition's free dim.
    inp8 = np.ascontiguousarray(
        input.reshape(B, CT, P, NPIX).transpose(0, 2, 1, 3)
        .astype(ml_dtypes.float8_e4m3))
    c2hyb = _pack_hybrid(c2)
    nc = _get_nc()
    in_maps = [
        {"inp": inp8[i], "c2h": c2hyb[i], **w}
        for i in range(B)
    ]
    res = run_bass_kernel_spmd(nc, in_maps, list(range(B)), trace=_trace)
    outp = np.stack([_unpack_hybrid(res.results[i]["outp"]) for i in range(B)])
    if _trace:
        kernel._last_result = res
    return outp


# revision 38
# speedup vs baseline: 1.1379x; 1.0256x over previous
"""Trainium2 Bass kernel for nn_BAMM (pooled self-attention block + residual).

Reference computation (per batch sample, B=8 sharded 1/core over 8 cores):
  x  = avg_pool4(input)          [512, 32, 32] -> flat [512, 1024]
  y  = avg_pool4(c2)
  q  = Wq @ x + bq               [128, 1024]
  k  = Wk @ y + bk               [128, 1024]
  v  = Wv @ y + bv               [512, 1024]
  E  = (q^T k) / sqrt(128)       [1024, 1024]
  A  = softmax(E, axis=-1)
  o  = v @ A^T                   [512, 1024]
  out = upsample4(o) + c2        [512, 128, 128]

I/O strategy (HBM traffic is the roofline: 358 GB/s per core):
  - input is shipped as fp8 (e3m4) in natural [C, H*W] layout: 8 MB read.
    It only feeds q, whose influence on the output is tiny (attention
    deviations contribute ~2e-4 to an output of unit scale), so 4 mantissa
    bits are far more than enough.
  - c2 is shipped as bf16 in a "pool-plane hybrid" layout
    [C, nb(8), k(16), n128] where k = (i,j) indexes the 16 elements of each
    4x4 pooling window and n128 = (a, w') the 128 pooled positions of block
    nb. One read (16 MB); the full tensor stays resident in SBUF so the
    residual needs no second read.
  - out is written bf16 in the same hybrid layout (16 MB); the host
    un-permutes and upcasts. Total 40 MB/core vs 120 MB for the f32 version.

Compute mapping (v3: every PE matmul pays ~88 ns of serial LDWEIGHTS, so
minimize matmul count and keep pooling off the PE):
  - q: pooling fused into the PE: 16 offset-strided accumulating matmuls of
    512 columns per channel tile read the full-res fp8 input directly (wq
    has scale/16 folded in).
  - y-pool: contiguous bf16 tensor_tensor fold-adds on DVE (2x mode;
    k-planes are contiguous in the hybrid layout), at nb-pair granularity.
  - attention: eT[m,n] = k_blk^T q per m-block chases the c2 stream;
    softmax denominators via ones-matmul column sums; exp on ACT (energies
    are O(0.1), no max subtraction needed). bv is folded in after the
    normalize as a per-partition ACT bias (o += bv * colsum * recip = bv).
  - residual: out-plane[k] = c2-plane[k] + onrm, contiguous TT adds per
    (channel tile, half), in place in the SBUF cache, then 2 MB stores.
"""

import sys
import types

import numpy as np

import bass_rust

import concourse.bass as bass
import concourse.tile as tile
from concourse import mybir
from concourse.bass_utils import run_bass_kernel_spmd
from concourse.vector_clock import ScopedClock


class _TileContextPatched(tile.TileContext):
    """Work around a walrus sync-wait-count limit: the stock kernel-tail
    InstDrain carries every outstanding sem wait; this walrus build rejects
    more than one sync wait on a Drain. Spread the surplus across nofuse NOPs.
    """

    def _drain_and_barrier(self, tick_clock, wait_clock):
        nc = self.nc
        drain_inst = nc.sync.drain()
        wait_clock.add_sem_waits(
            drain_inst.ins, ScopedClock({None: tick_clock.global_clock})
        )
        si = drain_inst.ins.sync_info
        if si is not None and si.on_wait and len(si.on_wait) > 1:
            waits = list(si.on_wait)
            si.on_wait = waits[:1]
            for i in range(1, len(waits)):
                nop = nc.sync.nop(nofuse=True)
                nop.ins.sync_info = bass_rust.SyncInfo(
                    on_wait=waits[i:i + 1], on_update=[]
                )

        nc.all_engine_barrier()
        assert self.sems is not None
        popped = nc._tile_sem_poison_stack.pop()
        assert popped is self._sem_poison
        nc.clear_and_free_semaphores(list(self.sems.allocated().values()))
        nc.all_engine_barrier()

F32 = mybir.dt.float32
BF16 = mybir.dt.bfloat16
FP8 = mybir.dt.float8e4  # e4m3 (DoubleRow perf mode requires e4/e5)
QALPHA = float(2 ** 14)  # q rescale so fp8 wq lands in e4m3's normal range

_MW_COUNTER = [0]


def _split_multi_waits(nc, max_waits=1):
    """This walrus build encodes at most one sync wait per instruction.
    Hoist surplus waits onto same-engine NoOps inserted just before the
    over-subscribed instruction (engine programs execute in order, so the
    NoOps block the engine until every wait is satisfied)."""
    for f in nc.m.functions:
        for bb in f.blocks:
            new_list = []
            for ins in bb.instructions:
                si = ins.sync_info
                if si is not None and si.on_wait and len(si.on_wait) > max_waits:
                    waits = list(si.on_wait)
                    extras, keep = waits[:-max_waits], waits[-max_waits:]
                    for w in extras:
                        _MW_COUNTER[0] += 1
                        nop = bass_rust.InstNoOp(
                            name=f"I-mw{_MW_COUNTER[0]}", engine=ins.engine
                        )
                        nop.sync_info = bass_rust.SyncInfo(
                            on_wait=[w], on_update=[]
                        )
                        new_list.append(nop)
                    si.on_wait = keep
                new_list.append(ins)
            bb.instructions[:] = new_list

P = 128          # partitions
C = 512          # channels
CT = C // P      # 4 channel tiles
H = 128          # input spatial
DS = 4           # pool factor
NPIX = H * H     # 16384
N2 = 1024        # pooled positions
NB = 8           # position blocks of 128
MT = 8           # m-tiles (= NB)
CK = 128         # q/k channels


def _install_ntff_shim():
    """Register the axon NTFF profile hook if the image's antenv lacks it."""
    try:
        import antenv.axon_hooks  # noqa: F401
        return
    except ImportError:
        pass
    try:
        from trn_agent_boot.trn_boot import _ntff_profile_via_ctypes
        hook = _ntff_profile_via_ctypes("/opt/axon/libaxon_pjrt.so")
        m = types.ModuleType("antenv.axon_hooks")
        m.get_axon_ntff_profile_hook = lambda: hook
        sys.modules["antenv.axon_hooks"] = m
    except Exception:
        pass


def build_nc(split_waits=True):
    nc = bass.Bass()

    inp = nc.declare_dram_parameter("inp", [P, CT, NPIX], FP8, isOutput=False)
    c2h = nc.declare_dram_parameter("c2h", [C, NPIX], BF16, isOutput=False)
    # host-preprocessed weights (see prep_weights for the scale folding)
    wq = nc.declare_dram_parameter("wq", [P, CT, CK], FP8, isOutput=False)
    wk = nc.declare_dram_parameter("wk", [C, CK], BF16, isOutput=False)
    wv = nc.declare_dram_parameter("wv", [C, C], BF16, isOutput=False)
    bq = nc.declare_dram_parameter("bq", [CK, 1], F32, isOutput=False)  # * scale
    bk = nc.declare_dram_parameter("bk", [CK, 1], F32, isOutput=False)
    bv4 = nc.declare_dram_parameter("bv4", [P, CT], F32, isOutput=False)
    out = nc.declare_dram_parameter("outp", [C, NPIX], BF16, isOutput=True)

    with _TileContextPatched(nc) as tc:
        with nc.allow_low_precision(
            reason="bf16/fp8 attention path sits far inside the 2e-2 gate"
        ):
            _emit(nc, tc, inp, c2h, wq, wk, wv, bq, bk, bv4, out)
    if split_waits:
        _split_multi_waits(nc)
    return nc


def _emit(nc, tc, inp, c2h, wq, wk, wv, bq, bk, bv4, out):
    from contextlib import ExitStack

    Exp = mybir.ActivationFunctionType.Exp

    ctx = ExitStack()
    with ctx:
        const = ctx.enter_context(tc.tile_pool(name="const", bufs=1))
        feat = ctx.enter_context(tc.tile_pool(name="feat", bufs=1))
        stream = ctx.enter_context(tc.tile_pool(name="stream", bufs=3))
        psum = ctx.enter_context(tc.tile_pool(name="psum", bufs=1, space="PSUM"))

        # ---- constants: wq/bq must be dispatched ahead of everything (the
        # q matmuls start within a few us); the rest are not needed until
        # the attention phase.
        wq_sb = [const.tile([P, 2, CK], FP8, tag=f"wq{i}", name=f"wq{i}")
                 for i in range(2)]
        wk_sb = [const.tile([P, CK], BF16, tag=f"wk{i}", name=f"wk{i}") for i in range(CT)]
        wv_sb = [const.tile([P, C], BF16, tag=f"wv{i}", name=f"wv{i}") for i in range(CT)]
        bq_sb = const.tile([P, 1], F32, tag="bq")
        bk_sb = const.tile([P, 1], F32, tag="bk")
        bv_sb = const.tile([P, CT], F32, tag="bv")
        ones_sb = const.tile([P, P], BF16, tag="ones")
        for g in range(2):
            nc.scalar.dma_start(out=wq_sb[g][:], in_=wq[:, 2 * g:2 * g + 2, :])
        nc.scalar.dma_start(out=bq_sb[:], in_=bq[:])
        nc.vector.memset(ones_sb[:], 1.0)

        # ---- persistent tiles ----
        c2call = feat.tile([P, CT * NPIX], BF16, tag="c2c")
        c2c = [c2call[:, i * NPIX:(i + 1) * NPIX] for i in range(CT)]
        yf = [feat.tile([P, N2], BF16, tag=f"yf{i}", name=f"yf{i}") for i in range(CT)]
        q_sb = feat.tile([P, N2], BF16, tag="q")
        k_sb = feat.tile([P, N2], BF16, tag="k")
        vt_sb = [feat.tile([P, C], BF16, tag=f"vt{i}", name=f"vt{i}") for i in range(MT)]
        et_sb = [feat.tile([P, N2], BF16, tag=f"et{i}", name=f"et{i}") for i in range(MT)]
        recip = feat.tile([P, N2], F32, tag="recip")

        for i in range(CT):
            nc.scalar.dma_start(out=wk_sb[i][:], in_=wk[i * P:(i + 1) * P, :])
            nc.scalar.dma_start(out=wv_sb[i][:], in_=wv[i * P:(i + 1) * P, :])
        nc.scalar.dma_start(out=bk_sb[:], in_=bk[:])
        nc.scalar.dma_start(out=bv_sb[:], in_=bv4[:, :])

        # ---- inp stream + fused pool-q matmuls (SP ring), with k/v blocks
        # interleaved between q-chunk groups so the post-q PE chain is just
        # energies + column sums.
        # s outer / ct inner so each 512-col psum accumulation group finishes
        # before the next group in the same bank starts (the start flag
        # clears has_written bits bank-wide).
        qp = psum.tile([P, N2], F32, tag="big", bufs=1)

        def emit_k(nb):
            blk = slice(nb * P, (nb + 1) * P)
            kp = psum.tile([P, 512], F32, tag="small", bufs=1, name="kp")
            for ct in range(CT):
                nc.tensor.matmul(kp[:, 0:128], wk_sb[ct][:], yf[ct][:, blk],
                                 start=(ct == 0), stop=(ct == CT - 1))
            nc.scalar.add(k_sb[:, blk], kp[:, 0:128], bk_sb[:])

        def emit_v(nb):
            blk = slice(nb * P, (nb + 1) * P)
            vp = psum.tile([P, 512], F32, tag="vt", bufs=1, name="vp")
            for ct in range(CT):
                nc.tensor.matmul(vp[:], yf[ct][:, blk], wv_sb[ct][:],
                                 start=(ct == 0), stop=(ct == CT - 1))
            nc.scalar.copy(out=vt_sb[nb][:], in_=vp[:])

        for s in range(4):
            for g in range(2):
                ich = stream.tile([P, 8192], FP8, tag="ich", bufs=2, name="ich")
                # stripe across the SP HWDGE and GPSIMD SWDGE rings: each
                # ring executes its DMAs FIFO (~250 GB/s ceiling), so one
                # ring alone cannot reach the 358 GB/s HBM rate.
                eng = nc.sync if (s * 2 + g) % 2 == 0 else nc.gpsimd
                eng.dma_start(
                    out=ich[:],
                    in_=inp[:, 2 * g:2 * g + 2, s * 4096:(s + 1) * 4096])
                v = ich[:].rearrange("p (t a i w j) -> p t i j a w",
                                     t=2, a=8, i=4, w=32, j=4)
                for ij in range(16):
                    i4, j4 = ij // 4, ij % 4
                    nc.tensor.matmul(
                        qp[:, s * 256:(s + 1) * 256], wq_sb[g][:],
                        v[:, :, i4, j4, :, :],
                        perf_mode=mybir.MatmulPerfMode.DoubleRow,
                        start=(g == 0 and ij == 0),
                        stop=(g == 1 and ij == 15),
                        skip_group_check=True)
        nc.scalar.add(q_sb[:], qp[:], bq_sb[:])

        # ---- c2 cache loads: same SP HWDGE ring as the inp chunks, queued
        # BEHIND them so the small inp stream drains at full HBM rate first
        # (q gates all attention work); the slab queue then streams at line
        # rate with no sequencer compute behind it.
        for nbp in range(4):
            cols = slice(nbp * 4096, (nbp + 1) * 4096)
            for ct in range(CT):
                eng = nc.sync if ct % 2 == 0 else nc.gpsimd
                eng.dma_start(out=c2c[ct][:, cols],
                              in_=c2h[ct * P:(ct + 1) * P, cols])

        # ---- c2 pooling: DVE fold-adds at nb-pair granularity, emitted
        # first so the DVE chases the slab DMAs with no PE dependency.
        for nbp in range(4):
            for ct in range(CT):
                sl2 = c2c[ct].rearrange("p (b x) -> p b x", b=NB)
                s3 = sl2[:, 2 * nbp:2 * nbp + 2, :]
                ftmp = feat.tile([P, 2048], BF16, tag="ftmp", bufs=2,
                                 name="ftmp")
                f2 = ftmp[:].rearrange("p (b x) -> p b x", b=2)
                nc.vector.tensor_add(f2[:, :, :], s3[:, :, 0:1024],
                                     s3[:, :, 1024:2048])
                nc.vector.tensor_add(f2[:, :, 0:512], f2[:, :, 0:512],
                                     f2[:, :, 512:1024])
                nc.vector.tensor_add(f2[:, :, 0:256], f2[:, :, 0:256],
                                     f2[:, :, 256:512])
                yv = yf[ct][:, nbp * 256:(nbp + 1) * 256].rearrange(
                    "p (b x) -> p b x", b=2)
                nc.vector.tensor_add(yv, f2[:, :, 0:128], f2[:, :, 128:256])

        # ---- attention phase: per position-block k/v -> energy -> exp ->
        # colsum, all chasing the c2 stream (q is done early).
        sp = [psum.tile([P, 512], F32, tag=f"sp{nh}", name=f"sp{nh}", bufs=1)
              for nh in range(2)]

        def emit_colsums(nb):
            for nh in range(2):
                nc.tensor.matmul(
                    sp[nh][:], ones_sb[:],
                    et_sb[nb][:, nh * 512:(nh + 1) * 512],
                    start=(nb == 0), stop=(nb == NB - 1),
                    skip_group_check=True)

        pend = None
        for nb in range(NB):
            blk = slice(nb * P, (nb + 1) * P)
            emit_k(nb)
            if pend is not None:
                emit_colsums(pend)
                pend = None
            epb = psum.tile([P, N2], F32, tag="ep", bufs=1, name="epb")
            nc.tensor.matmul(epb[:, 0:512], k_sb[:, blk], q_sb[:, 0:512],
                             start=True, stop=True)
            nc.tensor.matmul(epb[:, 512:1024], k_sb[:, blk], q_sb[:, 512:1024],
                             start=True, stop=True)
            nc.scalar.activation(out=et_sb[nb][:], in_=epb[:], func=Exp)
            emit_v(nb)
            pend = nb
        emit_colsums(pend)

        # 1/x as exp(-ln(x)) on the (idle) ACT engine: ~2 us total vs ~7 us
        # for DVE's iterative-divide reciprocal, and off the DVE tail path.
        for nh in range(2):
            lnp = psum.tile([P, 512], F32, tag="small", bufs=1, name="lnp")
            nc.scalar.activation(out=lnp[:], in_=sp[nh][:],
                                 func=mybir.ActivationFunctionType.Ln)
            nc.scalar.activation(out=recip[:, nh * 512:(nh + 1) * 512],
                                 in_=lnp[:], func=Exp, scale=-1.0)

        # ---- tail: out bmm, normalize (+bv), residual + upsample, store ----
        for ct in range(CT):
            ops = psum.tile([P, N2], F32, tag="big", bufs=1, name="ops")
            for mt in range(MT):
                for nh in range(2):
                    nc.tensor.matmul(
                        ops[:, nh * 512:(nh + 1) * 512],
                        vt_sb[mt][:, ct * P:(ct + 1) * P],
                        et_sb[mt][:, nh * 512:(nh + 1) * 512],
                        start=(mt == 0), stop=(mt == MT - 1),
                        skip_group_check=True)
            onrm = feat.tile([P, N2], BF16, tag="onrm", bufs=2, name="onrm")
            for nh in range(2):
                half = slice(nh * 512, (nh + 1) * 512)
                nc.vector.tensor_mul(onrm[:, half], ops[:, half], recip[:, half])
                nc.scalar.add(onrm[:, half], onrm[:, half], bv_sb[:, ct:ct + 1])
                ov = (onrm[:, half].rearrange("p (b n) -> p b n", b=4)
                      .unsqueeze(2).broadcast_to([P, 4, 16, P]))
                cols = slice(nh * 8192, (nh + 1) * 8192)
                cv = c2c[ct][:, cols].rearrange("p (b k n) -> p b k n",
                                                b=4, k=16)
                nc.vector.tensor_add(cv, cv, ov)
                # stripe the out stream across the ACT and SP HWDGE rings
                # (both are idle by the tail) to beat the per-ring ceiling
                eng = nc.scalar if nh == 0 else nc.sync
                eng.dma_start(out=out[ct * P:(ct + 1) * P, cols],
                              in_=c2c[ct][:, cols])


_NC_CACHE = None


def _get_nc():
    global _NC_CACHE
    if _NC_CACHE is None:
        _install_ntff_shim()
        _NC_CACHE = build_nc()
    return _NC_CACHE


def prep_weights(Wq, bq, Wk, bk, Wv, bv):
    scale = np.float32(1.0 / np.sqrt(np.float32(CK)))
    sixteenth = np.float32(1.0 / 16.0)
    alpha = np.float32(QALPHA)
    import ml_dtypes
    bf16 = ml_dtypes.bfloat16
    fp8 = ml_dtypes.float8_e4m3
    # q is computed at alpha x scale so the fp8 wq values sit in e4m3's
    # normal range; k (and bk) carry the compensating 1/alpha.
    wq8 = (Wq.T * (scale * sixteenth * alpha)).astype(fp8)      # [C, CK]
    wq8 = wq8.reshape(CT, P, CK).transpose(1, 0, 2)             # [P, CT, CK]
    return {
        "wq": np.ascontiguousarray(wq8),
        "wk": np.ascontiguousarray((Wk.T * (sixteenth / alpha)).astype(bf16)),
        "wv": np.ascontiguousarray((Wv.T * sixteenth).astype(bf16)),
        "bq": np.ascontiguousarray((bq * (scale * alpha)).reshape(CK, 1),
                                   dtype=np.float32),
        "bk": np.ascontiguousarray((bk / alpha).reshape(CK, 1),
                                   dtype=np.float32),
        "bv4": np.ascontiguousarray(bv.reshape(CT, P).T, dtype=np.float32),
    }


def _pack_hybrid(x):
    """[B, C, 128, 128] f32 -> [B, C, 16384] bf16 in [nb, (i,j), (a,w')] order."""
    import ml_dtypes
    B = x.shape[0]
    xb = x.reshape(B, C, NB, 4, 4, 32, 4)          # [b, c, nb, a, i, w', j]
    xb = xb.transpose(0, 1, 2, 4, 6, 3, 5)         # [b, c, nb, i, j, a, w']
    return np.ascontiguousarray(xb.reshape(B, C, NPIX).astype(ml_dtypes.bfloat16))


def _unpack_hybrid(y):
    """[C, 16384] bf16 hybrid -> [C, 128, 128] f32 natural."""
    yb = np.asarray(y).astype(np.float32).reshape(C, NB, 4, 4, 4, 32)
    # dims: [c, nb, i, j, a, w'] -> [c, nb, a, i, w', j]
    yb = yb.transpose(0, 1, 4, 2, 5, 3)
    return yb.reshape(C, H, H)


def kernel(input, c2, Wq, bq, Wk, bk, Wv, bv, _trace=False):
    import ml_dtypes
    input = np.asarray(input, dtype=np.float32)
    c2 = np.asarray(c2, dtype=np.float32)
    w = prep_weights(
        np.asarray(Wq, np.float32), np.asarray(bq, np.float32),
        np.asarray(Wk, np.float32), np.asarray(bk, np.float32),
        np.asarray(Wv, np.float32), np.asarray(bv, np.float32),
    )
    B = input.shape[0]
    # [B, C, npix] -> [B, P, CT, npix]: partition p holds channels
    # {p, 128+p, 256+p, 384+p} so DoubleRow matmuls contract channel-tile
    # pairs out of one partition's free dim.
    inp8 = np.ascontiguousarray(
        input.reshape(B, CT, P, NPIX).transpose(0, 2, 1, 3)
        .astype(ml_dtypes.float8_e4m3))
    c2hyb = _pack_hybrid(c2)
    nc = _get_nc()
    in_maps = [
        {"inp": inp8[i], "c2h": c2hyb[i], **w}
        for i in range(B)
    ]
    res = run_bass_kernel_spmd(nc, in_maps, list(range(B)), trace=_trace)
    outp = np.stack([_unpack_hybrid(res.results[i]["outp"]) for i in range(B)])
    if _trace:
        kernel._last_result = res
    return outp
